# revision 25
# baseline (speedup 1.0000x reference)
"""Trainium2 Bass kernel for nn_CrossModalFusion (B=8, C=512, H=W=64, Td=512).

Math notes (exact, not approximations):
  * The MHA uses a length-1 key/value sequence, so softmax over that axis is
    exactly 1.0 and the attention output is the V projection of the text
    vector broadcast over all L = H*W query positions.  The Q/K projections
    never influence the output.
  * Therefore rgb_t/dep_t (and everything downstream: the gate conv input,
    BN stats, the gate, and `fused`) are constant over H and W:
        r[b] = Wo_r @ (Wv_r @ (tp_w @ feat[b] + tp_b) + bv_r) + bo_r
        d[b] = likewise with depth weights
        pre[b] = gate_w[:, :C] @ r[b] + gate_w[:, C:] @ d[b] + gate_b
        BN over batch only (spatial variance is 0), gate = sigmoid(...),
        fused[b, c, :, :] = gate[b, c] * r[b, c] + (1 - gate[b, c]) * d[b, c]
  * The big rgb_c4/depth_c4 tensors only feed the two scalar losses
    (mean squared diff and mean abs diff).

Sharding: data-parallel over batch, one batch element per core (8 cores).
Each core streams its rgb/depth element and accumulates per-partition loss
sums on-device; the tiny 8-batch linear chain + batch-norm is replicated on
every core (cheaper than collectives for 16 KB of activations), with the
text features column-rotated per core so column 0 is the core's own batch.
The fused output slice is written with a broadcast DMA (stride-0 source).

The host only does layout work (slicing wqkv, transposing weight matrices,
rotating feat columns) plus the final unshard: stacking the 8 fused slices
and summing the per-core loss partial sums.
"""

import os

import numpy as np

P = 128          # SBUF partitions
C = 512          # channels
HW = 64 * 64     # spatial positions per batch element
B = 8            # batch (== number of cores)
NCH = C // P     # channel chunks (4)
NKJ = 2 * C // P # gate_w contraction chunks (8)
BN_EPS = 1e-5

N_CORES = 8

_NC_CACHE = {}
LAST_RESULTS = None  # BassKernelResults of the most recent kernel() call


def _build_nc():
    """Build (once) the single-core Bass program run SPMD on all 8 cores."""
    import concourse.bacc as bacc
    import concourse.mybir as mybir
    import concourse.tile as tile

    f32 = mybir.dt.float32
    Act = mybir.ActivationFunctionType
    Alu = mybir.AluOpType

    nc = bacc.Bacc()

    # -------- I/O --------
    rgb = nc.dram_tensor("rgb", (C, HW), f32, kind="ExternalInput")
    dep = nc.dram_tensor("dep", (C, HW), f32, kind="ExternalInput")
    featT = nc.dram_tensor("featT", (C, B), f32, kind="ExternalInput")
    tp_wT = nc.dram_tensor("tp_wT", (C, C), f32, kind="ExternalInput")
    wvT_r = nc.dram_tensor("wvT_r", (C, C), f32, kind="ExternalInput")
    woT_r = nc.dram_tensor("woT_r", (C, C), f32, kind="ExternalInput")
    wvT_d = nc.dram_tensor("wvT_d", (C, C), f32, kind="ExternalInput")
    woT_d = nc.dram_tensor("woT_d", (C, C), f32, kind="ExternalInput")
    gwT = nc.dram_tensor("gwT", (2 * C, C), f32, kind="ExternalInput")
    # biases[(chunk, p, col)]: 0=tp_b 1=bv_r 2=bo_r 3=bv_d 4=bo_d 5=gate_b 6=bn_g 7=bn_b
    biases = nc.dram_tensor("biases", (NCH, P, 8), f32, kind="ExternalInput")

    fused_out = nc.dram_tensor("fused_out", (C, HW), f32, kind="ExternalOutput")
    # loss_out[:, 2*mc] = sum(diff^2) over chunk mc row, [:, 2*mc+1] = sum(|diff|)
    loss_out = nc.dram_tensor("loss_out", (P, 16), f32, kind="ExternalOutput")

    with tile.TileContext(nc) as tc:
        with (
            tc.tile_pool(name="weights", bufs=1) as wpool,
            tc.tile_pool(name="small", bufs=1) as spool,
            tc.tile_pool(name="stream", bufs=6) as stpool,
            tc.tile_pool(name="psum", bufs=4, space="PSUM") as ppool,
        ):
            # -------- small + weight loads (emitted first: chain needs them early)
            feat_sb = spool.tile([P, NCH, B], f32)
            nc.sync.dma_start(
                out=feat_sb, in_=featT[:, :].rearrange("(k p) b -> p k b", p=P)
            )
            bias_sb = spool.tile([P, NCH, 8], f32)
            nc.sync.dma_start(
                out=bias_sb, in_=biases[:, :, :].rearrange("k p c -> p k c")
            )

            # All nc.sync DMAs drain through ONE FIFO queue (sprayed over the
            # 16 SDMA engines at full BW), so emission order == transfer
            # order. Interleave weight chunks and stream chunks in deadline
            # order: tp chunks first (chain start), then each loss chunk's
            # data between successive weight stages.
            def load_w(dram, nk, tag):
                tiles = []
                for kc in range(nk):
                    t = wpool.tile([P, C], f32, tag=f"{tag}{kc}")
                    nc.sync.dma_start(out=t, in_=dram[kc * P : (kc + 1) * P, :])
                    tiles.append(t)
                return tiles

            # Weights first (the whole chain unblocks by ~28 us so the output
            # write overlaps the input stream), then the loss strips.
            w_tp = load_w(tp_wT, NCH, "w_tp")
            w_vr = load_w(wvT_r, NCH, "w_vr")
            w_or = load_w(woT_r, NCH, "w_or")
            w_vd = load_w(wvT_d, NCH, "w_vd")
            w_od = load_w(woT_d, NCH, "w_od")
            w_gw = load_w(gwT, NKJ, "w_gw")

            # Loss data in [P, 2048] strips (2 per channel chunk per modality)
            # for a fine-grained DMA->sub->abs->square pipeline.
            NST = 2 * NCH  # 8 strips per modality
            SW = HW // 2   # strip width
            rgb_tiles, dep_tiles = [], []
            for s in range(NST):
                mc, h = s // 2, s % 2
                cs = slice(h * SW, (h + 1) * SW)
                rgb_t = stpool.tile([P, SW], f32, tag="rgb_t")
                nc.sync.dma_start(out=rgb_t, in_=rgb[mc * P : (mc + 1) * P, cs])
                dep_t = stpool.tile([P, SW], f32, tag="dep_t")
                nc.sync.dma_start(out=dep_t, in_=dep[mc * P : (mc + 1) * P, cs])
                rgb_tiles.append(rgb_t)
                dep_tiles.append(dep_t)

            # loss partial sums per strip: col 2s = sum(diff^2), 2s+1 = sum|diff|
            loss_sb = spool.tile([P, 2 * NST], f32)

            def stream_chunk(s):
                rt, dt_ = rgb_tiles[s], dep_tiles[s]
                nc.vector.tensor_sub(rt, rt, dt_)          # diff (in place)
                nc.scalar.activation(                      # |diff| + row sum
                    out=rt, in_=rt, func=Act.Abs,
                    accum_out=loss_sb[:, 2 * s + 1 : 2 * s + 2],
                )
                nc.scalar.activation(                      # diff^2 + row sum
                    out=rt, in_=rt, func=Act.Square,
                    accum_out=loss_sb[:, 2 * s : 2 * s + 1],
                )

            # -------- tiny linear chain, batch-stationary orientation.
            # Activations ride as the PE stationary operand ([128, 8] chunks)
            # and the 512-wide weight tiles are the moving operand, so each
            # stage is 4 wide matmuls (fp32, 4 cyc/row) instead of 16 weight
            # loads — ~2x faster PE chain. Stage output [8, 512] is copied to
            # SBUF, transposed back to [128, 8] chunks via PE, and biases are
            # applied on the transposed evac.
            from concourse.masks import make_identity

            ident = spool.tile([P, P], f32)
            make_identity(nc, ident)

            y1 = spool.tile([P, NCH, B], f32)
            v_r = spool.tile([P, NCH, B], f32)
            r_r = spool.tile([P, NCH, B], f32)
            v_d = spool.tile([P, NCH, B], f32)
            r_d = spool.tile([P, NCH, B], f32)
            pre = spool.tile([P, NCH, B], f32)

            def mm_stage(dst, w_tiles, lhs_chunks, bias_col, nk):
                # row = sum_kc lhs_chunks[kc].T @ w_tiles[kc]  -> [B, C] psum
                psA = ppool.tile([B, C], f32, tag="psA")
                for kc in range(nk):
                    nc.tensor.matmul(
                        psA, lhs_chunks[kc], w_tiles[kc][:, :],
                        start=(kc == 0), stop=(kc == nk - 1),
                    )
                row = spool.tile([B, C], f32, tag="row")
                nc.vector.tensor_copy(row, psA)
                # transpose back to [128, 8] chunks and add the bias
                for mc in range(NCH):
                    psT = ppool.tile([P, B], f32, tag="psT")
                    nc.tensor.transpose(
                        psT, row[:, mc * P : (mc + 1) * P], ident[0:B, 0:B]
                    )
                    nc.vector.tensor_scalar_add(
                        dst[:, mc, :], psT, bias_sb[:, mc, bias_col : bias_col + 1]
                    )

            feat_chunks = [feat_sb[:, kc, :] for kc in range(NCH)]
            mm_stage(y1, w_tp, feat_chunks, 0, NCH)
            stream_chunk(0)
            y1_chunks = [y1[:, kc, :] for kc in range(NCH)]
            mm_stage(v_r, w_vr, y1_chunks, 1, NCH)
            stream_chunk(1)
            mm_stage(r_r, w_or, [v_r[:, kc, :] for kc in range(NCH)], 2, NCH)
            stream_chunk(2)
            mm_stage(v_d, w_vd, y1_chunks, 3, NCH)
            stream_chunk(3)
            mm_stage(r_d, w_od, [v_d[:, kc, :] for kc in range(NCH)], 4, NCH)
            stream_chunk(4)
            cat_chunks = [r_r[:, kc, :] for kc in range(NCH)] + [
                r_d[:, kc, :] for kc in range(NCH)
            ]
            mm_stage(pre, w_gw, cat_chunks, 5, NKJ)
            for s in range(5, NST):
                stream_chunk(s)

            # -------- batch-norm over the 8 batch columns, per channel.
            # Everything after bn_stats is batched over the 4 channel chunks
            # as [P, 4] ops to keep the post-chain latency low.
            stats = spool.tile([P, NCH, 6], f32)
            mv = spool.tile([P, NCH, 2], f32)  # (mean, biased var)
            for mc in range(NCH):
                nc.vector.bn_stats(out=stats[:, mc, :], in_=pre[:, mc, :])
                nc.vector.bn_aggr(out=mv[:, mc, :], in_=stats[:, mc, :])
            veps = spool.tile([P, NCH], f32)
            sq = spool.tile([P, NCH], f32)
            nc.vector.tensor_scalar_add(veps, mv[:, :, 1:2], BN_EPS)
            nc.scalar.activation(out=sq, in_=veps, func=Act.Sqrt)
            # rstd = 1/sqrt(veps), with one Newton step to cover the ACT Sqrt
            # LUT's loose error budget: r1 = r0 * (1.5 - 0.5 * veps * r0^2)
            r0 = spool.tile([P, NCH], f32)
            tnw = spool.tile([P, NCH], f32)
            rstd = spool.tile([P, NCH], f32)
            nc.vector.reciprocal(out=r0, in_=sq)
            nc.vector.tensor_mul(tnw, r0, r0)
            nc.vector.tensor_mul(tnw, tnw, veps)
            nc.vector.tensor_scalar(
                out=tnw, in0=tnw, scalar1=-0.5, scalar2=1.5,
                op0=Alu.mult, op1=Alu.add,
            )
            nc.vector.tensor_mul(rstd, r0, tnw)

            # gate and fused value, column 0 (= this core's batch) only
            xh = spool.tile([P, NCH], f32)
            gate = spool.tile([P, NCH], f32)
            fcol = spool.tile([P, NCH], f32)
            nc.vector.tensor_sub(xh, pre[:, :, 0:1], mv[:, :, 0:1])
            nc.vector.tensor_mul(xh, xh, rstd)
            nc.vector.tensor_mul(xh, xh, bias_sb[:, :, 6:7])   # * bn_g
            nc.vector.tensor_add(xh, xh, bias_sb[:, :, 7:8])   # + bn_b
            nc.scalar.activation(out=gate, in_=xh, func=Act.Sigmoid)
            nc.vector.tensor_sub(fcol, r_r[:, :, 0:1], r_d[:, :, 0:1])
            nc.vector.tensor_mul(fcol, gate, fcol)
            nc.vector.tensor_add(fcol, fcol, r_d[:, :, 0:1])

            # -------- fused output: replicate each chunk's column into a real
            # 2048-wide SBUF tile, then plain contiguous DMAs (split across
            # queues). Stride-0-source DMA measured ~2x slower on writes.
            zeros2k = spool.tile([P, 2048], f32)
            nc.vector.memset(zeros2k, 0.0)
            # Output DMAs go out through the (idle) TensorE queue so they
            # drain concurrently with the tail of the input stream on the
            # sync queue instead of behind it.
            with tc.tile_pool(name="outrep", bufs=2) as opool:
                for mc in range(NCH):
                    orep = opool.tile([P, 2048], f32, tag="orep")
                    with tc.high_priority():
                        nc.vector.tensor_scalar_add(
                            orep, zeros2k, fcol[:, mc : mc + 1]
                        )
                    for j in range(2):
                        cs = slice(j * 2048, (j + 1) * 2048)
                        nc.gpsimd.dma_start(
                            out=fused_out[mc * P : (mc + 1) * P, cs],
                            in_=orep,
                        )

                nc.gpsimd.dma_start(out=loss_out[:, :], in_=loss_sb)

    nc.finalize()
    return nc


def _get_nc():
    if "nc" not in _NC_CACHE:
        _NC_CACHE["nc"] = _build_nc()
    return _NC_CACHE["nc"]


def _prep_in_maps(inputs):
    f = lambda a: np.ascontiguousarray(np.asarray(a, dtype=np.float32))
    rgb_c4 = f(inputs["rgb_c4"]).reshape(B, C, HW)
    depth_c4 = f(inputs["depth_c4"]).reshape(B, C, HW)
    text_feat = f(inputs["text_feat"])

    tp_wT = f(np.asarray(inputs["tp_w"]).T)
    wvT_r = f(np.asarray(inputs["rgb_wqkv"])[2 * C : 3 * C].T)
    woT_r = f(np.asarray(inputs["rgb_wo"]).T)
    wvT_d = f(np.asarray(inputs["dep_wqkv"])[2 * C : 3 * C].T)
    woT_d = f(np.asarray(inputs["dep_wo"]).T)
    gwT = f(np.asarray(inputs["gate_w"]).T)

    vecs = [
        inputs["tp_b"],
        np.asarray(inputs["rgb_bqkv"])[2 * C : 3 * C],
        inputs["rgb_bo"],
        np.asarray(inputs["dep_bqkv"])[2 * C : 3 * C],
        inputs["dep_bo"],
        inputs["gate_b"],
        inputs["bn_g"],
        inputs["bn_b"],
    ]
    biases = f(np.stack([np.asarray(v) for v in vecs], axis=1).reshape(NCH, P, 8))

    shared = {
        "tp_wT": tp_wT, "wvT_r": wvT_r, "woT_r": woT_r,
        "wvT_d": wvT_d, "woT_d": woT_d, "gwT": gwT, "biases": biases,
    }
    in_maps = []
    for i in range(N_CORES):
        perm = np.roll(np.arange(B), -i)  # column 0 = this core's batch
        in_maps.append(
            dict(
                shared,
                rgb=rgb_c4[i],
                dep=depth_c4[i],
                featT=f(text_feat[perm].T),
            )
        )
    return in_maps


def kernel(**inputs):
    global LAST_RESULTS
    from concourse.bass_utils import run_bass_kernel_spmd

    nc = _get_nc()
    in_maps = _prep_in_maps(inputs)
    res = run_bass_kernel_spmd(nc, in_maps, core_ids=list(range(N_CORES)))
    LAST_RESULTS = res

    fused = np.stack(
        [res.results[i]["fused_out"] for i in range(N_CORES)]
    ).reshape(B, C, 64, 64)
    loss = np.stack([res.results[i]["loss_out"] for i in range(N_CORES)])
    n_total = float(B * C * HW)
    pixel_loss = np.float32(loss[:, :, 0::2].sum(dtype=np.float64) / n_total)
    depth_loss = np.float32(loss[:, :, 1::2].sum(dtype=np.float64) / n_total)
    return fused, pixel_loss, depth_loss


# revision 26
# speedup vs baseline: 1.0944x; 1.0944x over previous
"""Trainium2 Bass kernel for nn_CrossModalFusion (B=8, C=512, H=W=64, Td=512).

Math notes (exact, not approximations):
  * The MHA uses a length-1 key/value sequence, so softmax over that axis is
    exactly 1.0 and the attention output is the V projection of the text
    vector broadcast over all L = H*W query positions.  The Q/K projections
    never influence the output.
  * Therefore rgb_t/dep_t (and everything downstream: the gate conv input,
    BN stats, the gate, and `fused`) are constant over H and W:
        r[b] = Wo_r @ (Wv_r @ (tp_w @ feat[b] + tp_b) + bv_r) + bo_r
        d[b] = likewise with depth weights
        pre[b] = gate_w[:, :C] @ r[b] + gate_w[:, C:] @ d[b] + gate_b
        BN over batch only (spatial variance is 0), gate = sigmoid(...),
        fused[b, c, :, :] = gate[b, c] * r[b, c] + (1 - gate[b, c]) * d[b, c]
  * The big rgb_c4/depth_c4 tensors only feed the two scalar losses
    (mean squared diff and mean abs diff).

Sharding: data-parallel over batch, one batch element per core (8 cores).
Each core streams its rgb/depth element and accumulates per-partition loss
sums on-device; the tiny 8-batch linear chain + batch-norm is replicated on
every core (cheaper than collectives for 16 KB of activations), with the
text features column-rotated per core so column 0 is the core's own batch.
The fused output slice is written with a broadcast DMA (stride-0 source).

The host only does layout work (slicing wqkv, transposing weight matrices,
rotating feat columns) plus the final unshard: stacking the 8 fused slices
and summing the per-core loss partial sums.
"""

import os

import numpy as np

P = 128          # SBUF partitions
C = 512          # channels
HW = 64 * 64     # spatial positions per batch element
B = 8            # batch (== number of cores)
NCH = C // P     # channel chunks (4)
NKJ = 2 * C // P # gate_w contraction chunks (8)
BN_EPS = 1e-5

N_CORES = 8

_NC_CACHE = {}
LAST_RESULTS = None  # BassKernelResults of the most recent kernel() call


def _build_nc():
    """Build (once) the single-core Bass program run SPMD on all 8 cores."""
    import concourse.bacc as bacc
    import concourse.mybir as mybir
    import concourse.tile as tile

    f32 = mybir.dt.float32
    Act = mybir.ActivationFunctionType
    Alu = mybir.AluOpType

    nc = bacc.Bacc()

    # -------- I/O --------
    rgb = nc.dram_tensor("rgb", (C, HW), f32, kind="ExternalInput")
    dep = nc.dram_tensor("dep", (C, HW), f32, kind="ExternalInput")
    featT = nc.dram_tensor("featT", (C, B), f32, kind="ExternalInput")
    tp_wT = nc.dram_tensor("tp_wT", (C, C), f32, kind="ExternalInput")
    wvT_r = nc.dram_tensor("wvT_r", (C, C), f32, kind="ExternalInput")
    woT_r = nc.dram_tensor("woT_r", (C, C), f32, kind="ExternalInput")
    wvT_d = nc.dram_tensor("wvT_d", (C, C), f32, kind="ExternalInput")
    woT_d = nc.dram_tensor("woT_d", (C, C), f32, kind="ExternalInput")
    gwT = nc.dram_tensor("gwT", (2 * C, C), f32, kind="ExternalInput")
    # biases[(chunk, p, col)]: 0=tp_b 1=bv_r 2=bo_r 3=bv_d 4=bo_d 5=gate_b 6=bn_g 7=bn_b
    biases = nc.dram_tensor("biases", (NCH, P, 8), f32, kind="ExternalInput")

    fused_out = nc.dram_tensor("fused_out", (C, HW), f32, kind="ExternalOutput")
    # loss_out[:, 2*mc] = sum(diff^2) over chunk mc row, [:, 2*mc+1] = sum(|diff|)
    loss_out = nc.dram_tensor("loss_out", (P, 16), f32, kind="ExternalOutput")

    with tile.TileContext(nc) as tc:
        with (
            tc.tile_pool(name="weights", bufs=1) as wpool,
            tc.tile_pool(name="small", bufs=1) as spool,
            tc.tile_pool(name="stream", bufs=6) as stpool,
            tc.tile_pool(name="psum", bufs=4, space="PSUM") as ppool,
        ):
            # -------- small + weight loads (emitted first: chain needs them early)
            feat_sb = spool.tile([P, NCH, B], f32)
            nc.sync.dma_start(
                out=feat_sb, in_=featT[:, :].rearrange("(k p) b -> p k b", p=P)
            )
            bias_sb = spool.tile([P, NCH, 8], f32)
            nc.sync.dma_start(
                out=bias_sb, in_=biases[:, :, :].rearrange("k p c -> p k c")
            )

            # All nc.sync DMAs drain through ONE FIFO queue (sprayed over the
            # 16 SDMA engines at full BW), so emission order == transfer
            # order. Interleave weight chunks and stream chunks in deadline
            # order: tp chunks first (chain start), then each loss chunk's
            # data between successive weight stages.
            def load_w(dram, nk, tag):
                tiles = []
                for kc in range(nk):
                    t = wpool.tile([P, C], f32, tag=f"{tag}{kc}")
                    nc.sync.dma_start(out=t, in_=dram[kc * P : (kc + 1) * P, :])
                    tiles.append(t)
                return tiles

            # Weights first (the whole chain unblocks by ~28 us so the output
            # write overlaps the input stream), then the loss strips.
            w_tp = load_w(tp_wT, NCH, "w_tp")
            w_vr = load_w(wvT_r, NCH, "w_vr")
            w_or = load_w(woT_r, NCH, "w_or")
            w_vd = load_w(wvT_d, NCH, "w_vd")
            w_od = load_w(woT_d, NCH, "w_od")
            w_gw = load_w(gwT, NKJ, "w_gw")

            # Loss data in [P, 2048] strips (2 per channel chunk per modality)
            # for a fine-grained DMA->sub->abs->square pipeline.
            NST = 2 * NCH  # 8 strips per modality
            SW = HW // 2   # strip width
            rgb_tiles, dep_tiles = [], []
            for s in range(NST):
                mc, h = s // 2, s % 2
                cs = slice(h * SW, (h + 1) * SW)
                rgb_t = stpool.tile([P, SW], f32, tag="rgb_t")
                nc.sync.dma_start(out=rgb_t, in_=rgb[mc * P : (mc + 1) * P, cs])
                dep_t = stpool.tile([P, SW], f32, tag="dep_t")
                nc.sync.dma_start(out=dep_t, in_=dep[mc * P : (mc + 1) * P, cs])
                rgb_tiles.append(rgb_t)
                dep_tiles.append(dep_t)

            # loss partial sums per strip: col 2s = sum(diff^2), 2s+1 = sum|diff|
            loss_sb = spool.tile([P, 2 * NST], f32)

            def stream_chunk(s):
                rt, dt_ = rgb_tiles[s], dep_tiles[s]
                nc.vector.tensor_sub(rt, rt, dt_)          # diff (in place)
                nc.scalar.activation(                      # |diff| + row sum
                    out=rt, in_=rt, func=Act.Abs,
                    accum_out=loss_sb[:, 2 * s + 1 : 2 * s + 2],
                )
                nc.scalar.activation(                      # diff^2 + row sum
                    out=rt, in_=rt, func=Act.Square,
                    accum_out=loss_sb[:, 2 * s : 2 * s + 1],
                )

            # -------- tiny linear chain, batch-stationary orientation.
            # Activations ride as the PE stationary operand ([128, 8] chunks)
            # and the 512-wide weight tiles are the moving operand, so each
            # stage is 4 wide matmuls (fp32, 4 cyc/row) instead of 16 weight
            # loads — ~2x faster PE chain. Stage output [8, 512] is copied to
            # SBUF, transposed back to [128, 8] chunks via PE, and biases are
            # applied on the transposed evac.
            from concourse.masks import make_identity

            ident = spool.tile([P, P], f32)
            make_identity(nc, ident)

            y1 = spool.tile([P, NCH, B], f32)
            v_r = spool.tile([P, NCH, B], f32)
            r_r = spool.tile([P, NCH, B], f32)
            v_d = spool.tile([P, NCH, B], f32)
            r_d = spool.tile([P, NCH, B], f32)
            pre = spool.tile([P, NCH, B], f32)

            def mm_stage(dst, w_tiles, lhs_chunks, bias_col, nk):
                # row = sum_kc lhs_chunks[kc].T @ w_tiles[kc]  -> [B, C] psum
                psA = ppool.tile([B, C], f32, tag="psA")
                for kc in range(nk):
                    nc.tensor.matmul(
                        psA, lhs_chunks[kc], w_tiles[kc][:, :],
                        start=(kc == 0), stop=(kc == nk - 1),
                    )
                row = spool.tile([B, C], f32, tag="row")
                nc.vector.tensor_copy(row, psA)
                # transpose back to [128, 8] chunks and add the bias
                for mc in range(NCH):
                    psT = ppool.tile([P, B], f32, tag="psT")
                    nc.tensor.transpose(
                        psT, row[:, mc * P : (mc + 1) * P], ident[0:B, 0:B]
                    )
                    nc.vector.tensor_scalar_add(
                        dst[:, mc, :], psT, bias_sb[:, mc, bias_col : bias_col + 1]
                    )

            feat_chunks = [feat_sb[:, kc, :] for kc in range(NCH)]
            mm_stage(y1, w_tp, feat_chunks, 0, NCH)
            stream_chunk(0)
            y1_chunks = [y1[:, kc, :] for kc in range(NCH)]
            mm_stage(v_r, w_vr, y1_chunks, 1, NCH)
            stream_chunk(1)
            mm_stage(r_r, w_or, [v_r[:, kc, :] for kc in range(NCH)], 2, NCH)
            stream_chunk(2)
            mm_stage(v_d, w_vd, y1_chunks, 3, NCH)
            stream_chunk(3)
            mm_stage(r_d, w_od, [v_d[:, kc, :] for kc in range(NCH)], 4, NCH)
            stream_chunk(4)
            cat_chunks = [r_r[:, kc, :] for kc in range(NCH)] + [
                r_d[:, kc, :] for kc in range(NCH)
            ]
            mm_stage(pre, w_gw, cat_chunks, 5, NKJ)
            for s in range(5, NST):
                stream_chunk(s)

            # -------- batch-norm over the 8 batch columns, per channel.
            # Everything after bn_stats is batched over the 4 channel chunks
            # as [P, 4] ops to keep the post-chain latency low.
            stats = spool.tile([P, NCH, 6], f32)
            mv = spool.tile([P, NCH, 2], f32)  # (mean, biased var)
            for mc in range(NCH):
                nc.vector.bn_stats(out=stats[:, mc, :], in_=pre[:, mc, :])
                nc.vector.bn_aggr(out=mv[:, mc, :], in_=stats[:, mc, :])
            veps = spool.tile([P, NCH], f32)
            sq = spool.tile([P, NCH], f32)
            nc.vector.tensor_scalar_add(veps, mv[:, :, 1:2], BN_EPS)
            nc.scalar.activation(out=sq, in_=veps, func=Act.Sqrt)
            # rstd = 1/sqrt(veps), with one Newton step to cover the ACT Sqrt
            # LUT's loose error budget: r1 = r0 * (1.5 - 0.5 * veps * r0^2)
            r0 = spool.tile([P, NCH], f32)
            tnw = spool.tile([P, NCH], f32)
            rstd = spool.tile([P, NCH], f32)
            nc.vector.reciprocal(out=r0, in_=sq)
            nc.vector.tensor_mul(tnw, r0, r0)
            nc.vector.tensor_mul(tnw, tnw, veps)
            nc.vector.tensor_scalar(
                out=tnw, in0=tnw, scalar1=-0.5, scalar2=1.5,
                op0=Alu.mult, op1=Alu.add,
            )
            nc.vector.tensor_mul(rstd, r0, tnw)

            # gate and fused value, column 0 (= this core's batch) only
            xh = spool.tile([P, NCH], f32)
            gate = spool.tile([P, NCH], f32)
            fcol = spool.tile([P, NCH], f32)
            nc.vector.tensor_sub(xh, pre[:, :, 0:1], mv[:, :, 0:1])
            nc.vector.tensor_mul(xh, xh, rstd)
            nc.vector.tensor_mul(xh, xh, bias_sb[:, :, 6:7])   # * bn_g
            nc.vector.tensor_add(xh, xh, bias_sb[:, :, 7:8])   # + bn_b
            nc.scalar.activation(out=gate, in_=xh, func=Act.Sigmoid)
            nc.vector.tensor_sub(fcol, r_r[:, :, 0:1], r_d[:, :, 0:1])
            nc.vector.tensor_mul(fcol, gate, fcol)
            nc.vector.tensor_add(fcol, fcol, r_d[:, :, 0:1])

            # -------- fused output: replicate each chunk's column into a real
            # 2048-wide SBUF tile, then plain contiguous DMAs (split across
            # queues). Stride-0-source DMA measured ~2x slower on writes.
            zeros2k = spool.tile([P, 2048], f32)
            nc.vector.memset(zeros2k, 0.0)
            # Output DMAs go out through the (idle) TensorE queue so they
            # drain concurrently with the tail of the input stream on the
            # sync queue instead of behind it.
            with tc.tile_pool(name="outrep", bufs=2) as opool:
                for mc in range(NCH):
                    orep = opool.tile([P, 2048], f32, tag="orep")
                    with tc.high_priority():
                        nc.vector.tensor_scalar_add(
                            orep, zeros2k, fcol[:, mc : mc + 1]
                        )
                    for j in range(2):
                        cs = slice(j * 2048, (j + 1) * 2048)
                        nc.sync.dma_start(
                            out=fused_out[mc * P : (mc + 1) * P, cs],
                            in_=orep,
                        )

                nc.sync.dma_start(out=loss_out[:, :], in_=loss_sb)

    nc.finalize()
    return nc


def _get_nc():
    if "nc" not in _NC_CACHE:
        _NC_CACHE["nc"] = _build_nc()
    return _NC_CACHE["nc"]


def _prep_in_maps(inputs):
    f = lambda a: np.ascontiguousarray(np.asarray(a, dtype=np.float32))
    rgb_c4 = f(inputs["rgb_c4"]).reshape(B, C, HW)
    depth_c4 = f(inputs["depth_c4"]).reshape(B, C, HW)
    text_feat = f(inputs["text_feat"])

    tp_wT = f(np.asarray(inputs["tp_w"]).T)
    wvT_r = f(np.asarray(inputs["rgb_wqkv"])[2 * C : 3 * C].T)
    woT_r = f(np.asarray(inputs["rgb_wo"]).T)
    wvT_d = f(np.asarray(inputs["dep_wqkv"])[2 * C : 3 * C].T)
    woT_d = f(np.asarray(inputs["dep_wo"]).T)
    gwT = f(np.asarray(inputs["gate_w"]).T)

    vecs = [
        inputs["tp_b"],
        np.asarray(inputs["rgb_bqkv"])[2 * C : 3 * C],
        inputs["rgb_bo"],
        np.asarray(inputs["dep_bqkv"])[2 * C : 3 * C],
        inputs["dep_bo"],
        inputs["gate_b"],
        inputs["bn_g"],
        inputs["bn_b"],
    ]
    biases = f(np.stack([np.asarray(v) for v in vecs], axis=1).reshape(NCH, P, 8))

    shared = {
        "tp_wT": tp_wT, "wvT_r": wvT_r, "woT_r": woT_r,
        "wvT_d": wvT_d, "woT_d": woT_d, "gwT": gwT, "biases": biases,
    }
    in_maps = []
    for i in range(N_CORES):
        perm = np.roll(np.arange(B), -i)  # column 0 = this core's batch
        in_maps.append(
            dict(
                shared,
                rgb=rgb_c4[i],
                dep=depth_c4[i],
                featT=f(text_feat[perm].T),
            )
        )
    return in_maps


def kernel(**inputs):
    global LAST_RESULTS
    from concourse.bass_utils import run_bass_kernel_spmd

    nc = _get_nc()
    in_maps = _prep_in_maps(inputs)
    res = run_bass_kernel_spmd(nc, in_maps, core_ids=list(range(N_CORES)))
    LAST_RESULTS = res

    fused = np.stack(
        [res.results[i]["fused_out"] for i in range(N_CORES)]
    ).reshape(B, C, 64, 64)
    loss = np.stack([res.results[i]["loss_out"] for i in range(N_CORES)])
    n_total = float(B * C * HW)
    pixel_loss = np.float32(loss[:, :, 0::2].sum(dtype=np.float64) / n_total)
    depth_loss = np.float32(loss[:, :, 1::2].sum(dtype=np.float64) / n_total)
    return fused, pixel_loss, depth_loss
